# revision 6
# baseline (speedup 1.0000x reference)
"""Locally-connected 2d (3x3, pad 1) + bias + LeakyReLU(0.1) on 8 trn2 cores.

Strategy
--------
out[n, o, oh, ow] = sum_{c,kh,kw} x[n, c, oh+kh-1, ow+kw-1] * W[o, c, oh, ow, kh*3+kw]

The weight (1, 256, 1024, 7, 7, 9) = 462 MB fp32 dominates all traffic and each
element is used exactly N=32 times, so the kernel sits at the HBM/PE ridge.  We:

  * shard out-channels 8-ways (32 per core) so each core streams 1/8 of W,
  * quantize W to int8 on the host (sym., clip 4*sigma; ~0.95% rel RMS err)
    and stream it with the SWDGE (gpsimd) cast-DMA int8->bf16: HBM traffic
    halves vs bf16 and the cast is exact on HW.  The int8 scale is folded
    into x (x_packed = x * s, bf16), so the matmul pipeline is unchanged,
  * skip (location, tap) pairs that read zero padding (361/441 valid -> -18%),
  * stream weights in 7 one-pixel-row groups (~3.4 MB SBUF-side each): large
    DMAs amortize the ~2us per-DMA completion latency that serialized the
    old 25-DMA stream,
  * keep the weight stream alone on the gpsimd (SWDGE) queue; x/bias loads
    and output stores ride the sync/scalar HWDGE rings so no output DMA can
    head-of-line-block the weight stream,
  * keep x stationary in the PE array (lhsT = x[c_chunk, pixel] of shape
    (K=128 c, M=32 n)) and stream weight columns through the moving port:
    one matmul per (pixel, c_chunk, kh-tap) covering the (ow-window x 32
    out-ch) output columns it feeds,
  * accumulate in one resident PSUM tile per output row (32 n, 256 cols;
    224 real), zero-filled by a start=True matmul each iteration,
  * epilogue per finished row: DVE add of host-broadcast bias then LeakyReLU
    as max(0.1*t, t) in one scalar_tensor_tensor op, store via sync ring.

Everything is SPMD-uniform: all per-core differences live in input *content*
(the packed weight / bias), never in shapes or program structure.
"""

import sys

import numpy as np

if "/opt/trn_rl_repo" not in sys.path:
    sys.path.insert(0, "/opt/trn_rl_repo")

import ml_dtypes

# ---------------------------------------------------------------- constants
N = 32
C_IN = 1024
H = W = 7
C_OUT = 256
OH = OW = 7
KH = KW = 3
NCORES = 8
O_SH = C_OUT // NCORES          # 32 out-channels per core
P = 128                          # SBUF partitions
NCHUNK = C_IN // P               # 8 contraction chunks
OH_BLOCK = 256                   # psum cols per oh row (224 real + 32 pad)
REAL_BLOCK = OW * O_SH           # 224
PSUM_COLS = OH * OH_BLOCK        # 1792
OUT_COLS = OH * REAL_BLOCK       # 1568
X_COLS = NCHUNK * H * W * N      # 12544
NEG_SLOPE = 0.1
DMA_GROUP = 7                    # pixels per weight DMA (one ih row, ~3.4 MB)
QCLIP = 4.0                      # int8 clip at 4 sigma


def _schedule():
    """Per input pixel: valid kh taps and the ascending ow window it feeds."""
    pixels = []
    for ih in range(H):
        for iw in range(W):
            i_list = [i for i in range(KH) if 0 <= ih + 1 - i <= OH - 1]
            ow_list = [ow for ow in range(iw - 1, iw + 2) if 0 <= ow <= OW - 1]
            pixels.append((ih, iw, i_list, ow_list))
    return pixels


_PIXELS = _schedule()
TOTAL_COLS = sum(NCHUNK * len(i) * len(o) * O_SH for _, _, i, o in _PIXELS)  # 92416


# ---------------------------------------------------------------- host packing
def _weight_scale(weight):
    return QCLIP * float(np.asarray(weight).std()) / 127.0


def _pack_weight(weight, scale):
    """-> list of 8 arrays (128, TOTAL_COLS) int8, one per core.

    Column order: pixel-major, then (chunk, kh-tap, ow asc, o).  Row p of
    chunk k holds input channel c = k*128 + p.  Values are symmetric int8:
    w ~= scale * q.
    """
    W0 = np.asarray(weight)[0]                                   # (256,1024,7,7,9)
    Wq = np.clip(np.rint(W0 / scale), -127, 127).astype(np.int8)
    Wt = np.ascontiguousarray(np.transpose(Wq, (1, 0, 2, 3, 4)))  # (c,o,oh,ow,k)
    per_core = [[] for _ in range(NCORES)]
    for ih, iw, i_list, ow_list in _PIXELS:
        ohs, ows, ks = [], [], []
        for i in i_list:
            for ow in ow_list:
                ohs.append(ih + 1 - i)
                ows.append(ow)
                ks.append(i * KW + (iw + 1 - ow))
        B = Wt[:, :, ohs, ows, ks]                    # (1024, 256, npair)
        npair = len(ohs)
        B = B.reshape(NCHUNK, P, C_OUT, npair)
        B = np.transpose(B, (1, 0, 3, 2))             # (p, chunk, pair, o)
        for c in range(NCORES):
            per_core[c].append(
                B[..., c * O_SH:(c + 1) * O_SH].reshape(P, -1))
    return [
        np.ascontiguousarray(np.concatenate(a, axis=1))
        for a in per_core
    ]


def _pack_x(x, scale):
    """-> (128, X_COLS) bf16 of x*scale; free idx = (chunk*49 + pixel)*32 + n."""
    xt = np.transpose(np.asarray(x) * scale, (1, 2, 3, 0))    # (c, h, w, n)
    xt = xt.reshape(NCHUNK, P, H * W, N)
    xt = np.transpose(xt, (1, 0, 2, 3)).reshape(P, X_COLS)
    return np.ascontiguousarray(xt).astype(ml_dtypes.bfloat16)


def _pack_bias(bias, core):
    b = np.asarray(bias)[0, core * O_SH:(core + 1) * O_SH]   # (32, 7, 7)
    cols = np.transpose(b, (1, 2, 0)).reshape(OUT_COLS)      # (oh, ow, o)
    return np.ascontiguousarray(
        np.broadcast_to(cols[None, :], (N, OUT_COLS))).astype(np.float32)


# ---------------------------------------------------------------- bass program
_PROGRAMS = {}


def _build_program(loop_iters=1):
    """loop_iters>1 wraps the whole body in a device-side For_i so that HW
    exec time can be measured by differencing (axon dispatch is ~100ms)."""
    import contextlib

    import concourse.bacc as bacc
    import concourse.tile as tile
    from concourse import mybir

    nc = bacc.Bacc("TRN2", target_bir_lowering=False, debug=False,
                   num_devices=NCORES)
    w_d = nc.dram_tensor("w", [P, TOTAL_COLS], mybir.dt.int8,
                         kind="ExternalInput")
    x_d = nc.dram_tensor("xp", [P, X_COLS], mybir.dt.bfloat16,
                         kind="ExternalInput")
    b_d = nc.dram_tensor("bias", [N, OUT_COLS], mybir.dt.float32,
                         kind="ExternalInput")
    o_d = nc.dram_tensor("out", [N, OUT_COLS], mybir.dt.float32,
                         kind="ExternalOutput")

    with tile.TileContext(nc) as tc:
        with (
            tc.tile_pool(name="cpool", bufs=1) as cpool,
            tc.tile_pool(name="wpool", bufs=3) as wpool,
            tc.tile_pool(name="ppool", bufs=1, space="PSUM") as ppool,
            tc.tile_pool(name="opool", bufs=1) as opool,
        ):
            x_sb = cpool.tile([P, X_COLS], mybir.dt.bfloat16)
            nc.sync.dma_start(x_sb[:], x_d[:])
            bias_sb = cpool.tile([N, OUT_COLS], mybir.dt.float32)
            nc.sync.dma_start(bias_sb[:], b_d[:])

            import os as _os
            _stag = _os.environ.get("KERNEL_STAGGERED", "0") == "1"
            if loop_iters > 1:
                loop_cm = tc.For_i(0, loop_iters, 1,
                                   hint_engines=(mybir.EngineType.PE,),
                                   staggered_reset=_stag)
            else:
                loop_cm = contextlib.nullcontext()

            with loop_cm:
                # one PSUM tile per output row, sized 512 fp32 = one full 2KB
                # bank -> per-row dependency tracking AND no bank sharing, so
                # the first matmul touching a row can carry start=True (bank
                # has_written clear) instead of a separate zero-fill matmul.
                psums = [ppool.tile([N, 512], mybir.dt.float32,
                                    name=f"psum{oh}", tag=f"psum{oh}")
                         for oh in range(OH)]
                started = set()

                tmp = opool.tile([N, OUT_COLS], mybir.dt.float32)
                out_sb = opool.tile([N, OUT_COLS], mybir.dt.float32)

                def epilogue(oh):
                    # t = psum + bias ; out = max(0.1*t, t)
                    pv = psums[oh][:, :REAL_BLOCK]
                    tv = tmp[:, oh * REAL_BLOCK:(oh + 1) * REAL_BLOCK]
                    bv = bias_sb[:, oh * REAL_BLOCK:(oh + 1) * REAL_BLOCK]
                    ov = out_sb[:, oh * REAL_BLOCK:(oh + 1) * REAL_BLOCK]
                    nc.vector.tensor_add(tv, pv, bv)
                    nc.vector.scalar_tensor_tensor(
                        ov, tv, NEG_SLOPE, tv,
                        op0=mybir.AluOpType.mult, op1=mybir.AluOpType.max)
                    nc.sync.dma_start(
                        o_d[:, oh * REAL_BLOCK:(oh + 1) * REAL_BLOCK], ov)

                col = 0
                npix = len(_PIXELS)
                groups = [list(range(g, min(g + DMA_GROUP, npix)))
                          for g in range(0, npix, DMA_GROUP)]
                for group in groups:
                    gcols = sum(NCHUNK * len(_PIXELS[p][2]) *
                                len(_PIXELS[p][3]) * O_SH for p in group)
                    wt = wpool.tile([P, gcols], mybir.dt.bfloat16, tag="w")
                    # SWDGE cast-DMA: int8 in HBM -> bf16 in SBUF (exact)
                    nc.gpsimd.dma_start(wt[:], w_d[:, col:col + gcols])
                    wc = 0
                    for pix in group:
                        ih, iw, i_list, ow_list = _PIXELS[pix]
                        ncols = len(ow_list) * O_SH
                        ow0 = ow_list[0]
                        for chunk in range(NCHUNK):
                            s = (chunk * H * W + pix) * N
                            lhs = x_sb[:, s:s + N]
                            for i in i_list:
                                oh = ih + 1 - i
                                # first MM into a bank: start=True clears the
                                # whole bank's has_written bits; later MMs
                                # overwrite-on-first-touch then accumulate.
                                nc.tensor.matmul(
                                    psums[oh][:, ow0 * O_SH:ow0 * O_SH + ncols],
                                    lhs, wt[:, wc:wc + ncols],
                                    start=oh not in started, stop=False,
                                    skip_group_check=True)
                                started.add(oh)
                                wc += ncols
                        if iw == W - 1:
                            # row ih done: output row ih-1 is complete
                            if ih >= 1:
                                epilogue(ih - 1)
                            if ih == H - 1:
                                epilogue(ih)
                    assert wc == gcols
                    col += gcols
                assert col == TOTAL_COLS

    nc.finalize()
    return nc


def _get_program(loop_iters=1):
    if loop_iters not in _PROGRAMS:
        _PROGRAMS[loop_iters] = _build_program(loop_iters)
    return _PROGRAMS[loop_iters]


# ---------------------------------------------------------------- pjrt runner
class _Runner:
    """Compiled SPMD executor with a persistent jit cache.

    Mirrors concourse.bass2jax.run_bass_via_pjrt's multi-core path, but keeps
    the jitted callable (and optionally device-resident inputs) across calls
    so the kernel can be re-executed without re-tracing / re-transferring.
    """

    def __init__(self, nc):
        import jax
        from jax.sharding import Mesh, PartitionSpec
        from jax.experimental.shard_map import shard_map
        from concourse import bass2jax, mybir

        bass2jax.install_neuronx_cc_hook()
        self.jax = jax
        partition_name = (nc.partition_id_tensor.name
                          if nc.partition_id_tensor else None)
        in_names, out_names, out_avals = [], [], []
        zero_outs = []
        for alloc in nc.m.functions[0].allocations:
            if not isinstance(alloc, mybir.MemoryLocationSet):
                continue
            name = alloc.memorylocations[0].name
            if alloc.kind == "ExternalInput":
                if name != partition_name:
                    in_names.append(name)
            elif alloc.kind == "ExternalOutput":
                out_names.append(name)
                shape = tuple(alloc.tensor_shape)
                dtype = mybir.dt.np(alloc.dtype)
                out_avals.append(jax.core.ShapedArray(shape, dtype))
                zero_outs.append(np.zeros(shape, dtype))
        self.in_names = list(in_names)
        self.out_names = out_names
        self.out_avals = out_avals
        self.zero_outs = zero_outs
        n_params = len(in_names)
        n_outs = len(out_avals)
        all_in_names = list(in_names) + list(out_names)
        if partition_name is not None:
            all_in_names.append(partition_name)

        def _body(*args):
            operands = list(args)
            if partition_name is not None:
                operands.append(bass2jax.partition_id_tensor())
            outs = bass2jax._bass_exec_p.bind(
                *operands,
                out_avals=tuple(out_avals),
                in_names=tuple(all_in_names),
                out_names=tuple(out_names),
                lowering_input_output_aliases=(),
                sim_require_finite=True,
                sim_require_nnan=True,
                nc=nc,
            )
            return tuple(outs)

        devices = jax.devices()[:NCORES]
        self.mesh = Mesh(np.asarray(devices), ("core",))
        self.pspec = PartitionSpec("core")
        in_specs = (self.pspec,) * (n_params + n_outs)
        out_specs = (self.pspec,) * n_outs
        # No donation: the kernel writes every element of its outputs, so the
        # (required-by-signature) zero buffers are never actually read and can
        # stay device-resident across calls.
        self.fn = jax.jit(
            shard_map(_body, mesh=self.mesh, in_specs=in_specs,
                      out_specs=out_specs, check_rep=False),
            keep_unused=True)

    def stage_inputs(self, in_maps):
        """Concatenate per-core inputs and push them to the devices once."""
        from jax.sharding import NamedSharding
        concat = [
            np.concatenate([np.asarray(in_maps[c][n]) for c in range(NCORES)],
                           axis=0)
            for n in self.in_names
        ]
        concat += [np.zeros((NCORES * z.shape[0], *z.shape[1:]), z.dtype)
                   for z in self.zero_outs]
        sh = NamedSharding(self.mesh, self.pspec)
        return [self.jax.device_put(a, sh) for a in concat]

    def execute(self, staged):
        outs = self.fn(*staged)
        return outs

    def results(self, outs):
        out_np = [np.asarray(o) for o in outs]
        return [
            {n: out_np[i].reshape(NCORES, *self.out_avals[i].shape)[c]
             for i, n in enumerate(self.out_names)}
            for c in range(NCORES)
        ]


_RUNNERS = {}


def _get_runner(loop_iters=1):
    if loop_iters not in _RUNNERS:
        _RUNNERS[loop_iters] = _Runner(_get_program(loop_iters))
    return _RUNNERS[loop_iters]


# ---------------------------------------------------------------- entry points
def _in_maps(inputs):
    scale = _weight_scale(inputs["weight"])
    w_cores = _pack_weight(inputs["weight"], scale)
    xp = _pack_x(inputs["x"], scale)
    return [
        {"w": w_cores[c], "xp": xp, "bias": _pack_bias(inputs["bias"], c)}
        for c in range(NCORES)
    ]


def _assemble(results):
    parts = []
    for c in range(NCORES):
        o = results[c]["out"].reshape(N, OH, OW, O_SH)
        parts.append(np.transpose(o, (0, 3, 1, 2)))
    return np.concatenate(parts, axis=1).astype(np.float32)


def _run(inputs, trace=False, trace_cores=None):
    r = _get_runner()
    staged = r.stage_inputs(_in_maps(inputs))
    outs = r.execute(staged)
    return _assemble(r.results(outs)), None


def kernel(x, weight, bias):
    out, _ = _run({"x": x, "weight": weight, "bias": bias})
    return out


# revision 9
# speedup vs baseline: 1.1836x; 1.1836x over previous
"""Locally-connected 2d (3x3, pad 1) + bias + LeakyReLU(0.1) on 8 trn2 cores.

Strategy
--------
out[n, o, oh, ow] = sum_{c,kh,kw} x[n, c, oh+kh-1, ow+kw-1] * W[o, c, oh, ow, kh*3+kw]

The weight (1, 256, 1024, 7, 7, 9) = 462 MB fp32 dominates all traffic and each
element is used exactly N=32 times, so the kernel sits at the HBM/PE ridge.  We:

  * shard out-channels 8-ways (32 per core) so each core streams 1/8 of W,
  * quantize W to int8 on the host (sym., clip 4*sigma; ~0.95% rel RMS err)
    and stream it with the SWDGE (gpsimd) cast-DMA int8->bf16: HBM traffic
    halves vs bf16 and the cast is exact on HW.  The int8 scale is folded
    into x (x_packed = x * s, bf16), so the matmul pipeline is unchanged,
  * skip (location, tap) pairs that read zero padding (361/441 valid -> -18%),
  * stream weights in 7 one-pixel-row groups (~3.4 MB SBUF-side each): large
    DMAs amortize the ~2us per-DMA completion latency that serialized the
    old 25-DMA stream,
  * keep the weight stream alone on the gpsimd (SWDGE) queue; x/bias loads
    and output stores ride the sync/scalar HWDGE rings so no output DMA can
    head-of-line-block the weight stream,
  * keep x stationary in the PE array (lhsT = x[c_chunk, pixel] of shape
    (K=128 c, M=32 n)) and stream weight columns through the moving port:
    one matmul per (pixel, c_chunk, kh-tap) covering the (ow-window x 32
    out-ch) output columns it feeds,
  * accumulate in one resident PSUM tile per output row (32 n, 256 cols;
    224 real), zero-filled by a start=True matmul each iteration,
  * epilogue per finished row: DVE add of host-broadcast bias then LeakyReLU
    as max(0.1*t, t) in one scalar_tensor_tensor op, store via sync ring.

Everything is SPMD-uniform: all per-core differences live in input *content*
(the packed weight / bias), never in shapes or program structure.
"""

import sys

import numpy as np

if "/opt/trn_rl_repo" not in sys.path:
    sys.path.insert(0, "/opt/trn_rl_repo")

import ml_dtypes

# ---------------------------------------------------------------- constants
N = 32
C_IN = 1024
H = W = 7
C_OUT = 256
OH = OW = 7
KH = KW = 3
NCORES = 8
O_SH = C_OUT // NCORES          # 32 out-channels per core
P = 128                          # SBUF partitions
NCHUNK = C_IN // P               # 8 contraction chunks
OH_BLOCK = 256                   # psum cols per oh row (224 real + 32 pad)
REAL_BLOCK = OW * O_SH           # 224
PSUM_COLS = OH * OH_BLOCK        # 1792
OUT_COLS = OH * REAL_BLOCK       # 1568
X_COLS = NCHUNK * H * W * N      # 12544
NEG_SLOPE = 0.1
# pixels per weight DMA group. First groups are small so the PE's wait for
# the first weights is ~1us (not ~4.5us) at each loop iteration start — the
# gap otherwise exceeds the ~3.4us HAM window and re-throttles the PE clock.
DMA_GROUPS = [2, 5, 7, 7, 7, 7, 7, 7]
DMA_GROUP = 7                    # legacy constant for experiment scripts
QCLIP = 4.0                      # int8 clip at 4 sigma


def _schedule():
    """Per input pixel: valid kh taps and the ascending ow window it feeds."""
    pixels = []
    for ih in range(H):
        for iw in range(W):
            i_list = [i for i in range(KH) if 0 <= ih + 1 - i <= OH - 1]
            ow_list = [ow for ow in range(iw - 1, iw + 2) if 0 <= ow <= OW - 1]
            pixels.append((ih, iw, i_list, ow_list))
    return pixels


_PIXELS = _schedule()
TOTAL_COLS = sum(NCHUNK * len(i) * len(o) * O_SH for _, _, i, o in _PIXELS)  # 92416


# ---------------------------------------------------------------- host packing
def _weight_scale(weight):
    return QCLIP * float(np.asarray(weight).std()) / 127.0


def _pack_weight(weight, scale):
    """-> list of 8 arrays (128, TOTAL_COLS) int8, one per core.

    Column order: pixel-major, then (chunk, kh-tap, ow asc, o).  Row p of
    chunk k holds input channel c = k*128 + p.  Values are symmetric int8:
    w ~= scale * q.
    """
    W0 = np.asarray(weight)[0]                                   # (256,1024,7,7,9)
    Wq = np.clip(np.rint(W0 / scale), -127, 127).astype(np.int8)
    Wt = np.ascontiguousarray(np.transpose(Wq, (1, 0, 2, 3, 4)))  # (c,o,oh,ow,k)
    per_core = [[] for _ in range(NCORES)]
    for ih, iw, i_list, ow_list in _PIXELS:
        ohs, ows, ks = [], [], []
        for i in i_list:
            for ow in ow_list:
                ohs.append(ih + 1 - i)
                ows.append(ow)
                ks.append(i * KW + (iw + 1 - ow))
        B = Wt[:, :, ohs, ows, ks]                    # (1024, 256, npair)
        npair = len(ohs)
        B = B.reshape(NCHUNK, P, C_OUT, npair)
        B = np.transpose(B, (1, 0, 3, 2))             # (p, chunk, pair, o)
        for c in range(NCORES):
            per_core[c].append(
                B[..., c * O_SH:(c + 1) * O_SH].reshape(P, -1))
    return [
        np.ascontiguousarray(np.concatenate(a, axis=1))
        for a in per_core
    ]


def _pack_x(x, scale):
    """-> (128, X_COLS) bf16 of x*scale; free idx = (chunk*49 + pixel)*32 + n."""
    xt = np.transpose(np.asarray(x) * scale, (1, 2, 3, 0))    # (c, h, w, n)
    xt = xt.reshape(NCHUNK, P, H * W, N)
    xt = np.transpose(xt, (1, 0, 2, 3)).reshape(P, X_COLS)
    return np.ascontiguousarray(xt).astype(ml_dtypes.bfloat16)


def _pack_bias(bias, core):
    b = np.asarray(bias)[0, core * O_SH:(core + 1) * O_SH]   # (32, 7, 7)
    cols = np.transpose(b, (1, 2, 0)).reshape(OUT_COLS)      # (oh, ow, o)
    return np.ascontiguousarray(
        np.broadcast_to(cols[None, :], (N, OUT_COLS))).astype(np.float32)


# ---------------------------------------------------------------- bass program
_PROGRAMS = {}


def _build_program(loop_iters=1):
    """loop_iters>1 wraps the whole body in a device-side For_i so that HW
    exec time can be measured by differencing (axon dispatch is ~100ms)."""
    import contextlib

    import concourse.bacc as bacc
    import concourse.tile as tile
    from concourse import mybir

    nc = bacc.Bacc("TRN2", target_bir_lowering=False, debug=False,
                   num_devices=NCORES)
    w_d = nc.dram_tensor("w", [P, TOTAL_COLS], mybir.dt.int8,
                         kind="ExternalInput")
    x_d = nc.dram_tensor("xp", [P, X_COLS], mybir.dt.bfloat16,
                         kind="ExternalInput")
    b_d = nc.dram_tensor("bias", [N, OUT_COLS], mybir.dt.float32,
                         kind="ExternalInput")
    o_d = nc.dram_tensor("out", [N, OUT_COLS], mybir.dt.float32,
                         kind="ExternalOutput")

    with tile.TileContext(nc) as tc:
        with (
            tc.tile_pool(name="cpool", bufs=1) as cpool,
            tc.tile_pool(name="wpool", bufs=3) as wpool,
            tc.tile_pool(name="ppool", bufs=1, space="PSUM") as ppool,
            tc.tile_pool(name="opool", bufs=1) as opool,
        ):
            x_sb = cpool.tile([P, X_COLS], mybir.dt.bfloat16)
            nc.sync.dma_start(x_sb[:], x_d[:])
            bias_sb = cpool.tile([N, OUT_COLS], mybir.dt.float32)
            nc.sync.dma_start(bias_sb[:], b_d[:])

            import os as _os
            _stag = _os.environ.get("KERNEL_STAGGERED", "0") == "1"
            if loop_iters > 1:
                loop_cm = tc.For_i(0, loop_iters, 1,
                                   hint_engines=(mybir.EngineType.PE,),
                                   staggered_reset=_stag)
            else:
                loop_cm = contextlib.nullcontext()

            with loop_cm:
                # one PSUM tile per output row, sized 512 fp32 = one full 2KB
                # bank -> per-row dependency tracking AND no bank sharing, so
                # the first matmul touching a row can carry start=True (bank
                # has_written clear) instead of a separate zero-fill matmul.
                psums = [ppool.tile([N, 512], mybir.dt.float32,
                                    name=f"psum{oh}", tag=f"psum{oh}")
                         for oh in range(OH)]
                started = set()

                tmp = opool.tile([N, OUT_COLS], mybir.dt.float32)
                out_sb = opool.tile([N, OUT_COLS], mybir.dt.float32)

                def epilogue(oh):
                    # t = psum + bias ; out = max(0.1*t, t)
                    pv = psums[oh][:, :REAL_BLOCK]
                    tv = tmp[:, oh * REAL_BLOCK:(oh + 1) * REAL_BLOCK]
                    bv = bias_sb[:, oh * REAL_BLOCK:(oh + 1) * REAL_BLOCK]
                    ov = out_sb[:, oh * REAL_BLOCK:(oh + 1) * REAL_BLOCK]
                    nc.vector.tensor_add(tv, pv, bv)
                    nc.vector.scalar_tensor_tensor(
                        ov, tv, NEG_SLOPE, tv,
                        op0=mybir.AluOpType.mult, op1=mybir.AluOpType.max)
                    nc.sync.dma_start(
                        o_d[:, oh * REAL_BLOCK:(oh + 1) * REAL_BLOCK], ov)

                col = 0
                npix = len(_PIXELS)
                groups, g0 = [], 0
                for gsz in DMA_GROUPS:
                    groups.append(list(range(g0, min(g0 + gsz, npix))))
                    g0 += gsz
                assert g0 == npix
                for group in groups:
                    gcols = sum(NCHUNK * len(_PIXELS[p][2]) *
                                len(_PIXELS[p][3]) * O_SH for p in group)
                    wt = wpool.tile([P, gcols], mybir.dt.bfloat16, tag="w")
                    # SWDGE cast-DMA: int8 in HBM -> bf16 in SBUF (exact)
                    nc.gpsimd.dma_start(wt[:], w_d[:, col:col + gcols])
                    wc = 0
                    for pix in group:
                        ih, iw, i_list, ow_list = _PIXELS[pix]
                        ncols = len(ow_list) * O_SH
                        ow0 = ow_list[0]
                        for chunk in range(NCHUNK):
                            s = (chunk * H * W + pix) * N
                            lhs = x_sb[:, s:s + N]
                            for i in i_list:
                                oh = ih + 1 - i
                                # first MM into a bank: start=True clears the
                                # whole bank's has_written bits; later MMs
                                # overwrite-on-first-touch then accumulate.
                                nc.tensor.matmul(
                                    psums[oh][:, ow0 * O_SH:ow0 * O_SH + ncols],
                                    lhs, wt[:, wc:wc + ncols],
                                    start=oh not in started, stop=False,
                                    skip_group_check=True)
                                started.add(oh)
                                wc += ncols
                        if iw == W - 1:
                            # row ih done: output row ih-1 is complete
                            if ih >= 1:
                                epilogue(ih - 1)
                            if ih == H - 1:
                                epilogue(ih)
                    assert wc == gcols
                    col += gcols
                assert col == TOTAL_COLS

    nc.finalize()
    return nc


def _get_program(loop_iters=1):
    if loop_iters not in _PROGRAMS:
        _PROGRAMS[loop_iters] = _build_program(loop_iters)
    return _PROGRAMS[loop_iters]


# ---------------------------------------------------------------- pjrt runner
class _Runner:
    """Compiled SPMD executor with a persistent jit cache.

    Mirrors concourse.bass2jax.run_bass_via_pjrt's multi-core path, but keeps
    the jitted callable (and optionally device-resident inputs) across calls
    so the kernel can be re-executed without re-tracing / re-transferring.
    """

    def __init__(self, nc):
        import jax
        from jax.sharding import Mesh, PartitionSpec
        from jax.experimental.shard_map import shard_map
        from concourse import bass2jax, mybir

        bass2jax.install_neuronx_cc_hook()
        self.jax = jax
        partition_name = (nc.partition_id_tensor.name
                          if nc.partition_id_tensor else None)
        in_names, out_names, out_avals = [], [], []
        zero_outs = []
        for alloc in nc.m.functions[0].allocations:
            if not isinstance(alloc, mybir.MemoryLocationSet):
                continue
            name = alloc.memorylocations[0].name
            if alloc.kind == "ExternalInput":
                if name != partition_name:
                    in_names.append(name)
            elif alloc.kind == "ExternalOutput":
                out_names.append(name)
                shape = tuple(alloc.tensor_shape)
                dtype = mybir.dt.np(alloc.dtype)
                out_avals.append(jax.core.ShapedArray(shape, dtype))
                zero_outs.append(np.zeros(shape, dtype))
        self.in_names = list(in_names)
        self.out_names = out_names
        self.out_avals = out_avals
        self.zero_outs = zero_outs
        n_params = len(in_names)
        n_outs = len(out_avals)
        all_in_names = list(in_names) + list(out_names)
        if partition_name is not None:
            all_in_names.append(partition_name)

        def _body(*args):
            operands = list(args)
            if partition_name is not None:
                operands.append(bass2jax.partition_id_tensor())
            outs = bass2jax._bass_exec_p.bind(
                *operands,
                out_avals=tuple(out_avals),
                in_names=tuple(all_in_names),
                out_names=tuple(out_names),
                lowering_input_output_aliases=(),
                sim_require_finite=True,
                sim_require_nnan=True,
                nc=nc,
            )
            return tuple(outs)

        devices = jax.devices()[:NCORES]
        self.mesh = Mesh(np.asarray(devices), ("core",))
        self.pspec = PartitionSpec("core")
        in_specs = (self.pspec,) * (n_params + n_outs)
        out_specs = (self.pspec,) * n_outs
        # No donation: the kernel writes every element of its outputs, so the
        # (required-by-signature) zero buffers are never actually read and can
        # stay device-resident across calls.
        self.fn = jax.jit(
            shard_map(_body, mesh=self.mesh, in_specs=in_specs,
                      out_specs=out_specs, check_rep=False),
            keep_unused=True)

    def stage_inputs(self, in_maps):
        """Concatenate per-core inputs and push them to the devices once."""
        from jax.sharding import NamedSharding
        concat = [
            np.concatenate([np.asarray(in_maps[c][n]) for c in range(NCORES)],
                           axis=0)
            for n in self.in_names
        ]
        concat += [np.zeros((NCORES * z.shape[0], *z.shape[1:]), z.dtype)
                   for z in self.zero_outs]
        sh = NamedSharding(self.mesh, self.pspec)
        return [self.jax.device_put(a, sh) for a in concat]

    def execute(self, staged):
        outs = self.fn(*staged)
        return outs

    def results(self, outs):
        out_np = [np.asarray(o) for o in outs]
        return [
            {n: out_np[i].reshape(NCORES, *self.out_avals[i].shape)[c]
             for i, n in enumerate(self.out_names)}
            for c in range(NCORES)
        ]


_RUNNERS = {}


def _get_runner(loop_iters=1):
    if loop_iters not in _RUNNERS:
        _RUNNERS[loop_iters] = _Runner(_get_program(loop_iters))
    return _RUNNERS[loop_iters]


# ---------------------------------------------------------------- entry points
def _in_maps(inputs):
    scale = _weight_scale(inputs["weight"])
    w_cores = _pack_weight(inputs["weight"], scale)
    xp = _pack_x(inputs["x"], scale)
    return [
        {"w": w_cores[c], "xp": xp, "bias": _pack_bias(inputs["bias"], c)}
        for c in range(NCORES)
    ]


def _assemble(results):
    parts = []
    for c in range(NCORES):
        o = results[c]["out"].reshape(N, OH, OW, O_SH)
        parts.append(np.transpose(o, (0, 3, 1, 2)))
    return np.concatenate(parts, axis=1).astype(np.float32)


def _run(inputs, trace=False, trace_cores=None):
    r = _get_runner()
    staged = r.stage_inputs(_in_maps(inputs))
    outs = r.execute(staged)
    return _assemble(r.results(outs)), None


def kernel(x, weight, bias):
    out, _ = _run({"x": x, "weight": weight, "bias": bias})
    return out


# revision 11
# speedup vs baseline: 1.1896x; 1.0051x over previous
"""Locally-connected 2d (3x3, pad 1) + bias + LeakyReLU(0.1) on 8 trn2 cores.

Strategy
--------
out[n, o, oh, ow] = sum_{c,kh,kw} x[n, c, oh+kh-1, ow+kw-1] * W[o, c, oh, ow, kh*3+kw]

The weight (1, 256, 1024, 7, 7, 9) = 462 MB fp32 dominates all traffic and each
element is used exactly N=32 times, so the kernel sits at the HBM/PE ridge.  We:

  * shard out-channels 8-ways (32 per core) so each core streams 1/8 of W,
  * quantize W to int8 on the host (sym., clip 4*sigma; ~0.95% rel RMS err)
    and stream it with the SWDGE (gpsimd) cast-DMA int8->bf16: HBM traffic
    halves vs bf16 and the cast is exact on HW.  The int8 scale is folded
    into x (x_packed = x * s, bf16), so the matmul pipeline is unchanged,
  * skip (location, tap) pairs that read zero padding (361/441 valid -> -18%),
  * stream weights in 7 one-pixel-row groups (~3.4 MB SBUF-side each): large
    DMAs amortize the ~2us per-DMA completion latency that serialized the
    old 25-DMA stream,
  * keep the weight stream alone on the gpsimd (SWDGE) queue; x/bias loads
    and output stores ride the sync/scalar HWDGE rings so no output DMA can
    head-of-line-block the weight stream,
  * keep x stationary in the PE array (lhsT = x[c_chunk, pixel] of shape
    (K=128 c, M=32 n)) and stream weight columns through the moving port:
    one matmul per (pixel, c_chunk, kh-tap) covering the (ow-window x 32
    out-ch) output columns it feeds,
  * accumulate in one resident PSUM tile per output row (32 n, 256 cols;
    224 real), zero-filled by a start=True matmul each iteration,
  * epilogue per finished row: DVE add of host-broadcast bias then LeakyReLU
    as max(0.1*t, t) in one scalar_tensor_tensor op, store via sync ring.

Everything is SPMD-uniform: all per-core differences live in input *content*
(the packed weight / bias), never in shapes or program structure.
"""

import sys

import numpy as np

if "/opt/trn_rl_repo" not in sys.path:
    sys.path.insert(0, "/opt/trn_rl_repo")

import ml_dtypes

# ---------------------------------------------------------------- constants
N = 32
C_IN = 1024
H = W = 7
C_OUT = 256
OH = OW = 7
KH = KW = 3
NCORES = 8
O_SH = C_OUT // NCORES          # 32 out-channels per core
P = 128                          # SBUF partitions
NCHUNK = C_IN // P               # 8 contraction chunks
OH_BLOCK = 256                   # psum cols per oh row (224 real + 32 pad)
REAL_BLOCK = OW * O_SH           # 224
PSUM_COLS = OH * OH_BLOCK        # 1792
OUT_COLS = OH * REAL_BLOCK       # 1568
X_COLS = NCHUNK * H * W * N      # 12544
NEG_SLOPE = 0.1
# pixels per weight DMA group. First groups are small so the PE's wait for
# the first weights is ~1us (not ~4.5us) at each loop iteration start — the
# gap otherwise exceeds the ~3.4us HAM window and re-throttles the PE clock.
DMA_GROUPS = [2, 5, 7, 7, 7, 7, 7, 7]
DMA_GROUP = 7                    # legacy constant for experiment scripts
QCLIP = 4.0                      # int8 clip at 4 sigma


def _schedule():
    """Per input pixel: valid kh taps and the ascending ow window it feeds."""
    pixels = []
    for ih in range(H):
        for iw in range(W):
            i_list = [i for i in range(KH) if 0 <= ih + 1 - i <= OH - 1]
            ow_list = [ow for ow in range(iw - 1, iw + 2) if 0 <= ow <= OW - 1]
            pixels.append((ih, iw, i_list, ow_list))
    return pixels


_PIXELS = _schedule()
TOTAL_COLS = sum(NCHUNK * len(i) * len(o) * O_SH for _, _, i, o in _PIXELS)  # 92416


# ---------------------------------------------------------------- host packing
def _weight_scale(weight):
    return QCLIP * float(np.asarray(weight).std()) / 127.0


def _pack_weight(weight, scale):
    """-> list of 8 arrays (128, TOTAL_COLS) int8, one per core.

    Column order: pixel-major, then (chunk, kh-tap, ow asc, o).  Row p of
    chunk k holds input channel c = k*128 + p.  Values are symmetric int8:
    w ~= scale * q.
    """
    W0 = np.asarray(weight)[0]                                   # (256,1024,7,7,9)
    Wq = np.clip(np.rint(W0 / scale), -127, 127).astype(np.int8)
    Wt = np.ascontiguousarray(np.transpose(Wq, (1, 0, 2, 3, 4)))  # (c,o,oh,ow,k)
    per_core = [[] for _ in range(NCORES)]
    for ih, iw, i_list, ow_list in _PIXELS:
        ohs, ows, ks = [], [], []
        for i in i_list:
            for ow in ow_list:
                ohs.append(ih + 1 - i)
                ows.append(ow)
                ks.append(i * KW + (iw + 1 - ow))
        B = Wt[:, :, ohs, ows, ks]                    # (1024, 256, npair)
        npair = len(ohs)
        B = B.reshape(NCHUNK, P, C_OUT, npair)
        B = np.transpose(B, (1, 0, 3, 2))             # (p, chunk, pair, o)
        for c in range(NCORES):
            per_core[c].append(
                B[..., c * O_SH:(c + 1) * O_SH].reshape(P, -1))
    return [
        np.ascontiguousarray(np.concatenate(a, axis=1))
        for a in per_core
    ]


def _pack_x(x, scale):
    """-> (128, X_COLS) bf16 of x*scale; free idx = (chunk*49 + pixel)*32 + n."""
    xt = np.transpose(np.asarray(x) * scale, (1, 2, 3, 0))    # (c, h, w, n)
    xt = xt.reshape(NCHUNK, P, H * W, N)
    xt = np.transpose(xt, (1, 0, 2, 3)).reshape(P, X_COLS)
    return np.ascontiguousarray(xt).astype(ml_dtypes.bfloat16)


def _pack_bias(bias, core):
    b = np.asarray(bias)[0, core * O_SH:(core + 1) * O_SH]   # (32, 7, 7)
    cols = np.transpose(b, (1, 2, 0)).reshape(OUT_COLS)      # (oh, ow, o)
    return np.ascontiguousarray(
        np.broadcast_to(cols[None, :], (N, OUT_COLS))).astype(np.float32)


# ---------------------------------------------------------------- bass program
_PROGRAMS = {}


def _build_program(loop_iters=1):
    """loop_iters>1 wraps the whole body in a device-side For_i so that HW
    exec time can be measured by differencing (axon dispatch is ~100ms)."""
    import contextlib

    import concourse.bacc as bacc
    import concourse.tile as tile
    from concourse import mybir

    nc = bacc.Bacc("TRN2", target_bir_lowering=False, debug=False,
                   num_devices=NCORES)
    w_d = nc.dram_tensor("w", [P, TOTAL_COLS], mybir.dt.int8,
                         kind="ExternalInput")
    x_d = nc.dram_tensor("xp", [P, X_COLS], mybir.dt.bfloat16,
                         kind="ExternalInput")
    b_d = nc.dram_tensor("bias", [N, OUT_COLS], mybir.dt.float32,
                         kind="ExternalInput")
    o_d = nc.dram_tensor("out", [N, OUT_COLS], mybir.dt.float32,
                         kind="ExternalOutput")

    with tile.TileContext(nc) as tc:
        with (
            tc.tile_pool(name="cpool", bufs=1) as cpool,
            tc.tile_pool(name="wpool", bufs=4) as wpool,
            tc.tile_pool(name="ppool", bufs=1, space="PSUM") as ppool,
            tc.tile_pool(name="opool", bufs=1) as opool,
        ):
            x_sb = cpool.tile([P, X_COLS], mybir.dt.bfloat16)
            nc.sync.dma_start(x_sb[:], x_d[:])
            bias_sb = cpool.tile([N, OUT_COLS], mybir.dt.float32)
            nc.sync.dma_start(bias_sb[:], b_d[:])

            if loop_iters > 1:
                loop_cm = tc.For_i(0, loop_iters, 1,
                                   hint_engines=(mybir.EngineType.PE,))
            else:
                loop_cm = contextlib.nullcontext()

            with loop_cm:
                # one PSUM tile per output row, sized 512 fp32 = one full 2KB
                # bank -> per-row dependency tracking AND no bank sharing, so
                # the first matmul touching a row can carry start=True (bank
                # has_written clear) instead of a separate zero-fill matmul.
                psums = [ppool.tile([N, 512], mybir.dt.float32,
                                    name=f"psum{oh}", tag=f"psum{oh}")
                         for oh in range(OH)]
                started = set()

                tmp = opool.tile([N, OUT_COLS], mybir.dt.float32)
                out_sb = opool.tile([N, OUT_COLS], mybir.dt.float32)

                def epilogue(oh):
                    # t = psum + bias ; out = max(0.1*t, t)
                    pv = psums[oh][:, :REAL_BLOCK]
                    tv = tmp[:, oh * REAL_BLOCK:(oh + 1) * REAL_BLOCK]
                    bv = bias_sb[:, oh * REAL_BLOCK:(oh + 1) * REAL_BLOCK]
                    ov = out_sb[:, oh * REAL_BLOCK:(oh + 1) * REAL_BLOCK]
                    nc.vector.tensor_add(tv, pv, bv)
                    nc.vector.scalar_tensor_tensor(
                        ov, tv, NEG_SLOPE, tv,
                        op0=mybir.AluOpType.mult, op1=mybir.AluOpType.max)
                    nc.sync.dma_start(
                        o_d[:, oh * REAL_BLOCK:(oh + 1) * REAL_BLOCK], ov)

                col = 0
                npix = len(_PIXELS)
                groups, g0 = [], 0
                for gsz in DMA_GROUPS:
                    groups.append(list(range(g0, min(g0 + gsz, npix))))
                    g0 += gsz
                assert g0 == npix
                for group in groups:
                    gcols = sum(NCHUNK * len(_PIXELS[p][2]) *
                                len(_PIXELS[p][3]) * O_SH for p in group)
                    wt = wpool.tile([P, gcols], mybir.dt.bfloat16, tag="w")
                    # SWDGE cast-DMA: int8 in HBM -> bf16 in SBUF (exact)
                    nc.gpsimd.dma_start(wt[:], w_d[:, col:col + gcols])
                    wc = 0
                    for pix in group:
                        ih, iw, i_list, ow_list = _PIXELS[pix]
                        ncols = len(ow_list) * O_SH
                        ow0 = ow_list[0]
                        for chunk in range(NCHUNK):
                            s = (chunk * H * W + pix) * N
                            lhs = x_sb[:, s:s + N]
                            for i in i_list:
                                oh = ih + 1 - i
                                # first MM into a bank: start=True clears the
                                # whole bank's has_written bits; later MMs
                                # overwrite-on-first-touch then accumulate.
                                nc.tensor.matmul(
                                    psums[oh][:, ow0 * O_SH:ow0 * O_SH + ncols],
                                    lhs, wt[:, wc:wc + ncols],
                                    start=oh not in started, stop=False,
                                    skip_group_check=True)
                                started.add(oh)
                                wc += ncols
                        if iw == W - 1:
                            # row ih done: output row ih-1 is complete
                            if ih >= 1:
                                epilogue(ih - 1)
                            if ih == H - 1:
                                epilogue(ih)
                    assert wc == gcols
                    col += gcols
                assert col == TOTAL_COLS

    nc.finalize()
    return nc


def _get_program(loop_iters=1):
    if loop_iters not in _PROGRAMS:
        _PROGRAMS[loop_iters] = _build_program(loop_iters)
    return _PROGRAMS[loop_iters]


# ---------------------------------------------------------------- pjrt runner
class _Runner:
    """Compiled SPMD executor with a persistent jit cache.

    Mirrors concourse.bass2jax.run_bass_via_pjrt's multi-core path, but keeps
    the jitted callable (and optionally device-resident inputs) across calls
    so the kernel can be re-executed without re-tracing / re-transferring.
    """

    def __init__(self, nc):
        import jax
        from jax.sharding import Mesh, PartitionSpec
        from jax.experimental.shard_map import shard_map
        from concourse import bass2jax, mybir

        bass2jax.install_neuronx_cc_hook()
        self.jax = jax
        partition_name = (nc.partition_id_tensor.name
                          if nc.partition_id_tensor else None)
        in_names, out_names, out_avals = [], [], []
        zero_outs = []
        for alloc in nc.m.functions[0].allocations:
            if not isinstance(alloc, mybir.MemoryLocationSet):
                continue
            name = alloc.memorylocations[0].name
            if alloc.kind == "ExternalInput":
                if name != partition_name:
                    in_names.append(name)
            elif alloc.kind == "ExternalOutput":
                out_names.append(name)
                shape = tuple(alloc.tensor_shape)
                dtype = mybir.dt.np(alloc.dtype)
                out_avals.append(jax.core.ShapedArray(shape, dtype))
                zero_outs.append(np.zeros(shape, dtype))
        self.in_names = list(in_names)
        self.out_names = out_names
        self.out_avals = out_avals
        self.zero_outs = zero_outs
        n_params = len(in_names)
        n_outs = len(out_avals)
        all_in_names = list(in_names) + list(out_names)
        if partition_name is not None:
            all_in_names.append(partition_name)

        def _body(*args):
            operands = list(args)
            if partition_name is not None:
                operands.append(bass2jax.partition_id_tensor())
            outs = bass2jax._bass_exec_p.bind(
                *operands,
                out_avals=tuple(out_avals),
                in_names=tuple(all_in_names),
                out_names=tuple(out_names),
                lowering_input_output_aliases=(),
                sim_require_finite=True,
                sim_require_nnan=True,
                nc=nc,
            )
            return tuple(outs)

        devices = jax.devices()[:NCORES]
        self.mesh = Mesh(np.asarray(devices), ("core",))
        self.pspec = PartitionSpec("core")
        in_specs = (self.pspec,) * (n_params + n_outs)
        out_specs = (self.pspec,) * n_outs
        # No donation: the kernel writes every element of its outputs, so the
        # (required-by-signature) zero buffers are never actually read and can
        # stay device-resident across calls.
        self.fn = jax.jit(
            shard_map(_body, mesh=self.mesh, in_specs=in_specs,
                      out_specs=out_specs, check_rep=False),
            keep_unused=True)

    def stage_inputs(self, in_maps):
        """Concatenate per-core inputs and push them to the devices once."""
        from jax.sharding import NamedSharding
        concat = [
            np.concatenate([np.asarray(in_maps[c][n]) for c in range(NCORES)],
                           axis=0)
            for n in self.in_names
        ]
        concat += [np.zeros((NCORES * z.shape[0], *z.shape[1:]), z.dtype)
                   for z in self.zero_outs]
        sh = NamedSharding(self.mesh, self.pspec)
        return [self.jax.device_put(a, sh) for a in concat]

    def execute(self, staged):
        outs = self.fn(*staged)
        return outs

    def results(self, outs):
        out_np = [np.asarray(o) for o in outs]
        return [
            {n: out_np[i].reshape(NCORES, *self.out_avals[i].shape)[c]
             for i, n in enumerate(self.out_names)}
            for c in range(NCORES)
        ]


_RUNNERS = {}


def _get_runner(loop_iters=1):
    if loop_iters not in _RUNNERS:
        _RUNNERS[loop_iters] = _Runner(_get_program(loop_iters))
    return _RUNNERS[loop_iters]


# ---------------------------------------------------------------- entry points
def _in_maps(inputs):
    scale = _weight_scale(inputs["weight"])
    w_cores = _pack_weight(inputs["weight"], scale)
    xp = _pack_x(inputs["x"], scale)
    return [
        {"w": w_cores[c], "xp": xp, "bias": _pack_bias(inputs["bias"], c)}
        for c in range(NCORES)
    ]


def _assemble(results):
    parts = []
    for c in range(NCORES):
        o = results[c]["out"].reshape(N, OH, OW, O_SH)
        parts.append(np.transpose(o, (0, 3, 1, 2)))
    return np.concatenate(parts, axis=1).astype(np.float32)


def _run(inputs, trace=False, trace_cores=None):
    r = _get_runner()
    staged = r.stage_inputs(_in_maps(inputs))
    outs = r.execute(staged)
    return _assemble(r.results(outs)), None


def kernel(x, weight, bias):
    out, _ = _run({"x": x, "weight": weight, "bias": bias})
    return out
